# revision 1
# baseline (speedup 1.0000x reference)
"""GCN layer (message passing) on 8 Trainium2 NeuronCores.

out = relu(((D^-1/2 A D^-1/2) X) @ W.T) + X

Strategy (dst-sharded graph partitioning):
  - Destination nodes sharded across 8 cores (12500 nodes each); every core
    holds the full feature table (random-access gather source) and computes
    its 12500 output rows; the host concatenates.
  - Host-side prep (index-space only): per-edge weight ns2 = norm[src]*norm[dst]
    (both rsqrt-degree norms folded into the edge weight); edges grouped by
    (dst tile of 128 nodes, src bucket of 25000 nodes, src) so each dst tile's
    sources are gathered with dma_gather (int16 indices => src buckets), with
    ascending addresses per stream for HBM locality.
  - Device, per dst tile: up to 4 dma_gather calls pull all edge source rows
    into X (the dominant memory traffic ~216MB/core). The segment-sum runs on
    the tensor engine as  zT[i,d] += X_c[e,i].T @ S_c[e,d]  where
    S_c[e,d] = (d == local_dst[e]) * ns2[e] is built with one fused
    tensor_scalar (is_equal then mult) against a constant iota row matrix.
    Then y[d,o] = zT.T @ W.T on the PE, ReLU on ACT, residual add on DVE.
  - num_idxs per gather is static per (tile, bucket) = max count over the 8
    cores (SPMD same-program constraint), so padding is only the cross-core
    spread (~5%); pad slots gather row 0 of the bucket and are annihilated by
    local_dst = -1 (one-hot row of zeros). Unwritten tail columns of X are
    killed the same way, but the first X pool slots are memzeroed once since
    0 * garbage-NaN would poison PSUM.
"""

import math

import numpy as np

import concourse.bacc as bacc
import concourse.mybir as mybir
from concourse.bass_utils import run_bass_kernel_spmd
from concourse.tile import TileContext

P = 128
N_CORES = 8
BUCKET_MAX = 25000  # int16 gather indices: bucket the node space


def _prepare(features, W, edge_src, edge_dst, n_cores=N_CORES, bucket_max=BUCKET_MAX):
    """Partition the graph by dst core / dst tile / src bucket."""
    features = np.asarray(features, dtype=np.float32)
    W = np.asarray(W, dtype=np.float32)
    edge_src = np.asarray(edge_src, dtype=np.int32)
    edge_dst = np.asarray(edge_dst, dtype=np.int32)

    n_nodes, d = features.shape
    assert d == P
    assert n_nodes % n_cores == 0
    npc = n_nodes // n_cores
    n_tiles = math.ceil(npc / P)
    rows_last = npc - (n_tiles - 1) * P
    nb = math.ceil(n_nodes / bucket_max)
    B = math.ceil(n_nodes / nb)
    assert B <= 32768

    degs = np.bincount(edge_dst, minlength=n_nodes).astype(np.float32)
    norm = 1.0 / np.sqrt(np.maximum(degs, 1.0), dtype=np.float32)
    ns2 = norm[edge_src] * norm[edge_dst]

    core_of = edge_dst // npc

    # first pass: per-core sorted edge lists and per-(tile,bucket) counts
    per_core = []
    counts_all = np.zeros((n_cores, n_tiles, nb), np.int64)
    for k in range(n_cores):
        sel = np.flatnonzero(core_of == k)
        src_k = edge_src[sel]
        ldst = edge_dst[sel] - k * npc
        tile_of = ldst // P
        bucket = src_k // B
        order = np.lexsort((src_k, bucket, tile_of))
        sel = sel[order]
        gid = tile_of[order] * nb + bucket[order]
        counts = np.bincount(gid, minlength=n_tiles * nb).reshape(n_tiles, nb)
        counts_all[k] = counts
        per_core.append((sel, gid, (ldst[order] % P).astype(np.float32)))

    # static per-(tile,bucket) gather sizes: max across cores
    n_tb = counts_all.max(axis=0)  # [n_tiles, nb]
    ct_tb = (n_tb + P - 1) // P  # chunks per (tile, bucket)
    C_t = ct_tb.sum(axis=1)  # chunks per tile
    icols_tb = (n_tb + 15) // 16  # int16 idx columns per (tile, bucket)
    icols_t = icols_tb.sum(axis=1)

    # column offsets in the packed DRAM arrays
    chunk_off_in_tile = np.cumsum(ct_tb, axis=1) - ct_tb  # [n_tiles, nb]
    icol_off_in_tile = np.cumsum(icols_tb, axis=1) - icols_tb
    ldns_col_off = np.concatenate([[0], np.cumsum(3 * C_t)])[:-1]  # per tile
    icol_off_tile = np.concatenate([[0], np.cumsum(icols_t)])[:-1]
    total_icols = int(icols_t.sum())
    total_ldns = int((3 * C_t).sum())

    layout = dict(
        n_nodes=n_nodes,
        npc=npc,
        n_tiles=n_tiles,
        rows_last=rows_last,
        nb=nb,
        B=B,
        n_tb=n_tb,
        ct_tb=ct_tb,
        C_t=C_t,
        icols_tb=icols_tb,
        chunk_off_in_tile=chunk_off_in_tile,
        icol_off_in_tile=icol_off_in_tile,
        ldns_col_off=ldns_col_off,
        icol_off_tile=icol_off_tile,
        total_icols=total_icols,
        total_ldns=total_ldns,
    )

    in_maps = []
    wt = np.ascontiguousarray(W.T)  # wt[i, o] = W[o, i]
    iotam = np.tile(np.arange(P, dtype=np.float32), (P, 1))
    for k in range(n_cores):
        sel, gid, ld_sorted = per_core[k]
        group_start = np.zeros(n_tiles * nb, np.int64)
        cnts = counts_all[k].reshape(-1)
        group_start[1:] = np.cumsum(cnts)[:-1]
        pos = np.arange(len(sel)) - group_start[gid]
        t_of = gid // nb
        b_of = gid % nb

        # idx array [16, total_icols] then replicated to 128 partitions
        idx16 = np.zeros((16, total_icols), np.int16)
        icol = icol_off_tile[t_of] + icol_off_in_tile[t_of, b_of] + pos // 16
        idx16[pos % 16, icol] = (edge_src[sel] - b_of * B).astype(np.int16)
        idxm = np.tile(idx16, (8, 1))

        # ldns array [128, total_ldns]: per tile [ld columns | ns columns]
        ldns = np.zeros((P, total_ldns), np.float32)
        # default ld = -1 in all ld column regions
        for t in range(n_tiles):
            ldns[:, ldns_col_off[t] : ldns_col_off[t] + C_t[t]] = -1.0
        cit = chunk_off_in_tile[t_of, b_of] + pos // P
        e_idx = pos % P
        ldns[e_idx, ldns_col_off[t_of] + cit] = ld_sorted
        ldns[e_idx, ldns_col_off[t_of] + C_t[t_of] + cit] = ns2[sel]
        ldns[e_idx, ldns_col_off[t_of] + 2 * C_t[t_of] + cit] = -ns2[sel]

        in_maps.append(
            {
                "feats": features,
                "idxm": np.ascontiguousarray(idxm),
                "ldns": np.ascontiguousarray(ldns),
                "wt": wt,
                "iotam": iotam,
                "resid": np.ascontiguousarray(features[k * npc : (k + 1) * npc]),
            }
        )
    return in_maps, layout


def _build_program(layout):
    f32 = mybir.dt.float32
    i16 = mybir.dt.int16
    n_nodes = layout["n_nodes"]
    npc = layout["npc"]
    n_tiles = layout["n_tiles"]
    rows_last = layout["rows_last"]
    nb = layout["nb"]
    B = layout["B"]
    n_tb = layout["n_tb"]
    ct_tb = layout["ct_tb"]
    C_t = layout["C_t"]
    icols_tb = layout["icols_tb"]
    chunk_off_in_tile = layout["chunk_off_in_tile"]
    icol_off_in_tile = layout["icol_off_in_tile"]
    ldns_col_off = layout["ldns_col_off"]
    icol_off_tile = layout["icol_off_tile"]
    Cmax = int(C_t.max())

    nc = bacc.Bacc(num_swdge_queues=4)
    feats = nc.declare_dram_parameter("feats", [n_nodes, P], f32, isOutput=False)
    idxm = nc.declare_dram_parameter(
        "idxm", [P, layout["total_icols"]], i16, isOutput=False
    )
    ldns = nc.declare_dram_parameter(
        "ldns", [P, layout["total_ldns"]], f32, isOutput=False
    )
    wt = nc.declare_dram_parameter("wt", [P, P], f32, isOutput=False)
    iotam = nc.declare_dram_parameter("iotam", [P, P], f32, isOutput=False)
    resid = nc.declare_dram_parameter("resid", [npc, P], f32, isOutput=False)
    out = nc.declare_dram_parameter("out", [npc, P], f32, isOutput=True)

    X_BUFS = 3
    with TileContext(nc) as tc:
        with (
            tc.tile_pool(name="const", bufs=1) as constp,
            tc.tile_pool(name="meta", bufs=3) as metap,
            tc.tile_pool(name="x", bufs=X_BUFS) as xp,
            tc.tile_pool(name="s", bufs=6) as sp,
            tc.tile_pool(name="zps", bufs=2, space="PSUM") as zpsp,
            tc.tile_pool(name="yps", bufs=2, space="PSUM") as ypsp,
            tc.tile_pool(name="post", bufs=3) as postp,
        ):
            wt_sb = constp.tile([P, P], f32)
            nc.sync.dma_start(out=wt_sb[:], in_=wt[:, :])
            iota_f = constp.tile([P, P], f32)
            nc.sync.dma_start(out=iota_f[:], in_=iotam[:, :])

            for t in range(n_tiles):
                Ct = int(C_t[t])
                icols = int(icols_tb[t].sum())
                mt_i = metap.tile([P, max(icols, 1)], i16, tag="mi")
                mt_ln = metap.tile([P, 3 * Ct], f32, tag="mldns")
                ic0 = int(icol_off_tile[t])
                nc.sync.dma_start(out=mt_i[:, :icols], in_=idxm[:, ic0 : ic0 + icols])
                lc0 = int(ldns_col_off[t])
                nc.sync.dma_start(out=mt_ln[:], in_=ldns[:, lc0 : lc0 + 3 * Ct])

                # X[e, c*128:(c+1)*128] = feats[gathered src of (chunk c, slot e)]
                X_full = xp.tile([P, Cmax * P], f32, tag="X")
                X = X_full[:, : Ct * P]
                for b in range(nb):
                    n_idx = int(n_tb[t, b])
                    if n_idx == 0:
                        continue
                    co = int(chunk_off_in_tile[t, b])
                    cb = int(ct_tb[t, b])
                    io = int(icol_off_in_tile[t, b])
                    icb = int(icols_tb[t, b])
                    if n_idx % P:
                        # the gather leaves partitions >= n_idx%128 of its
                        # last chunk unwritten; pre-zero that chunk so
                        # 0 * NaN can't poison the one-hot matmul (memzero
                        # bitcasts to uint32 - no NaN read path)
                        nc.scalar.memzero(X[:, (co + cb - 1) * P : (co + cb) * P])
                    nc.gpsimd.dma_gather(
                        out_ap=X[:, co * P : (co + cb) * P].rearrange(
                            "p (c e) -> p c e", e=P
                        ),
                        in_ap=feats[b * B : min((b + 1) * B, n_nodes), :],
                        idxs_ap=mt_i[:, io : io + icb],
                        num_idxs=n_idx,
                        num_idxs_reg=n_idx,
                        elem_size=P,
                        # single_packet concatenates the whole stream into one
                        # SDMA packet; the packet limit is 64 descriptors, and
                        # these calls emit ~70-90 per engine
                        single_packet=False,
                        # one SWDGE queue per bucket: queues run on distinct
                        # Q7 core pairs, parallelizing descriptor generation
                        queue_num=b % 4,
                    )

                z_ps = zpsp.tile([P, P], f32)
                for c in range(Ct):
                    S = sp.tile([P, P], f32, tag="S")
                    # split one-hot builds across DVE and ACT (nc.any piled
                    # all of them onto DVE: 2.9ms busy in the profile).
                    # ACT has no tensor_scalar; for integer iota/ld,
                    # relu(ns - ns*(ld-iota)^2) == (iota==ld)*ns exactly.
                    if c % 2 == 0:
                        nc.vector.tensor_scalar(
                            out=S[:],
                            in0=iota_f[:],
                            scalar1=mt_ln[:, c : c + 1],
                            scalar2=mt_ln[:, Ct + c : Ct + c + 1],
                            op0=mybir.AluOpType.is_equal,
                            op1=mybir.AluOpType.mult,
                        )
                    else:
                        t2 = sp.tile([P, P], f32, tag="T2")
                        nc.scalar.activation(
                            out=t2[:],
                            in_=iota_f[:],
                            func=mybir.ActivationFunctionType.Square,
                            bias=mt_ln[:, c : c + 1],
                            scale=-1.0,
                        )
                        nc.scalar.activation(
                            out=S[:],
                            in_=t2[:],
                            func=mybir.ActivationFunctionType.Relu,
                            bias=mt_ln[:, Ct + c : Ct + c + 1],
                            scale=mt_ln[:, 2 * Ct + c : 2 * Ct + c + 1],
                        )
                    # zT[i, d] += X_c[e, i].T @ S[e, d]
                    nc.tensor.matmul(
                        out=z_ps[:],
                        lhsT=X[:, c * P : (c + 1) * P],
                        rhs=S[:],
                        start=(c == 0),
                        stop=(c == Ct - 1),
                    )

                zT_sb = postp.tile([P, P], f32, tag="zT")
                nc.scalar.copy(out=zT_sb[:], in_=z_ps[:])
                y_ps = ypsp.tile([P, P], f32)
                # y[d, o] = zT[i, d].T @ wt[i, o]
                nc.tensor.matmul(
                    out=y_ps[:], lhsT=zT_sb[:], rhs=wt_sb[:], start=True, stop=True
                )

                rows = P if t < n_tiles - 1 else rows_last
                y_sb = postp.tile([P, P], f32, tag="y")
                nc.scalar.activation(
                    out=y_sb[:], in_=y_ps[:], func=mybir.ActivationFunctionType.Relu
                )
                res_sb = postp.tile([P, P], f32, tag="res")
                nc.sync.dma_start(
                    out=res_sb[:rows], in_=resid[t * P : t * P + rows, :]
                )
                o_sb = postp.tile([P, P], f32, tag="o")
                nc.vector.tensor_add(
                    out=o_sb[:rows], in0=y_sb[:rows], in1=res_sb[:rows]
                )
                nc.sync.dma_start(out=out[t * P : t * P + rows, :], in_=o_sb[:rows])
    nc.finalize()
    return nc


def _run(features, W, edge_src, edge_dst, trace=False, **spmd_kwargs):
    in_maps, layout = _prepare(features, W, edge_src, edge_dst)
    nc = _build_program(layout)
    br = run_bass_kernel_spmd(
        nc, in_maps, core_ids=list(range(N_CORES)), trace=trace, **spmd_kwargs
    )
    outs = [r["out"] for r in br.results]
    full = np.concatenate(outs, axis=0).astype(np.float32)
    return full, br


def kernel(features, W, edge_src, edge_dst):
    out, _ = _run(features, W, edge_src, edge_dst, trace=False)
    return out



# revision 5
# speedup vs baseline: 1.8404x; 1.8404x over previous
"""GCN layer (message passing) on 8 Trainium2 NeuronCores.

out = relu(((D^-1/2 A D^-1/2) X) @ W.T) + X

Strategy (dst-sharded, fp16 gather, desc-rate-optimized):
  - Destination nodes sharded across 8 cores (12500 each). Every core sees the
    full feature table in DRAM as the random-access gather source.
  - Host prep: h16 = fp16(features * rsqrt-degree-norm) folds the src-side
    D^-1/2 into the gathered rows (pure per-node elementwise + dtype pack);
    the dst-side D^-1/2 is a per-partition scale fused into the final ReLU
    activation. Edges are grouped by (dst tile of 128, src bucket of 25000,
    src ascending) purely in index space.
  - The hard resource on TRN2 for this problem is SWDGE descriptor
    generation: microbenchmarked at ~9 ns/descriptor/queue, 4 queues max
    (~445-468 desc/us aggregate), independent of descriptor size and
    single_packet. One gather descriptor per edge is unavoidable (random
    256B rows), so per-core floor = 400k descs ~= 860us. fp16 rows (256B)
    halve HBM traffic vs fp32 so the byte side stays far from the 358GB/s
    limit; 4 buckets rotate over the 4 SWDGE queues.
  - Per dst tile: gathered rows X [slots, 128] fp16; segment-sum on the PE
    as zT[i,d] += X_c[e,i].T @ S_c[e,d] with one-hot S_c = (iota == ld_c)
    built by a single DVE tensor_scalar(is_equal) per chunk against a
    constant iota matrix (fp16: integers <= 2048 exact). Pad slots carry
    ld=-1 -> zero one-hot row. X pool buffers are memzeroed on first use so
    un-gathered tail slots can never inject NaN (0*NaN) into PSUM.
  - Then y[d,o] = zT.T @ W.T (fp16), y = relu(norm_dst * y) on ACT (scale is
    a per-partition AP), residual add on DVE, store.
  - num_idxs per (tile,bucket) = max count over the 8 cores (SPMD same
    program); short cores pad the idx stream with idx 0 and ld=-1.
"""

import math

import numpy as np

import concourse.bacc as bacc
import concourse.mybir as mybir
from concourse.bass_utils import run_bass_kernel_spmd
from concourse.tile import TileContext

P = 128
N_CORES = 8
N_NODES = 100000
NPC = N_NODES // N_CORES  # 12500
NB = 4
B = 25000  # src bucket size; int16 idx
N_TILES = math.ceil(NPC / P)  # 98
ROWS_LAST = NPC - (N_TILES - 1) * P  # 84
X_BUFS = 4


def _prepare(features, W, edge_src, edge_dst):
    features = np.asarray(features, dtype=np.float32)
    W = np.asarray(W, dtype=np.float32)
    edge_src = np.asarray(edge_src, dtype=np.int32)
    edge_dst = np.asarray(edge_dst, dtype=np.int32)
    n_nodes, d = features.shape
    assert d == P and n_nodes == N_NODES

    degs = np.bincount(edge_dst, minlength=n_nodes).astype(np.float32)
    norm = 1.0 / np.sqrt(np.maximum(degs, 1.0), dtype=np.float32)
    h16 = (features * norm[:, None]).astype(np.float16)
    wt16 = np.ascontiguousarray(W.T).astype(np.float16)  # [i, o]
    iota16 = np.tile(np.arange(P, dtype=np.float16), (P, 1))

    core_of = edge_dst // NPC
    per_core_sorted = []
    counts_all = np.zeros((N_CORES, N_TILES, NB), np.int64)
    for k in range(N_CORES):
        sel = np.flatnonzero(core_of == k)
        src_k = edge_src[sel]
        ldst = edge_dst[sel] - k * NPC
        tile_of = ldst // P
        bucket = src_k // B
        order = np.lexsort((src_k, bucket, tile_of))
        sel = sel[order]
        gid = tile_of[order] * NB + bucket[order]
        counts_all[k] = np.bincount(gid, minlength=N_TILES * NB).reshape(
            N_TILES, NB
        )
        per_core_sorted.append((sel, gid))

    n_tb = counts_all.max(axis=0)  # [98, 4] static num_idxs
    c_tb = (n_tb + P - 1) // P  # chunks per (t, b)
    C_t = c_tb.sum(axis=1)  # chunks per tile
    Cmax = int(C_t.max())
    slot_off_tb = (np.cumsum(c_tb, axis=1) - c_tb) * P  # slot offset in tile
    chunk_off_t = np.concatenate([[0], np.cumsum(C_t)])[:-1]
    icols_tb = (n_tb + 15) // 16
    icol_off_in_t = np.cumsum(icols_tb, axis=1) - icols_tb
    icols_t = icols_tb.sum(axis=1)
    icol_off_t = np.concatenate([[0], np.cumsum(icols_t)])[:-1]
    total_icols = int(icols_t.sum())
    total_chunks = int(C_t.sum())

    layout = dict(
        n_tb=n_tb,
        c_tb=c_tb,
        C_t=C_t,
        Cmax=Cmax,
        slot_off_tb=slot_off_tb,
        chunk_off_t=chunk_off_t,
        icols_tb=icols_tb,
        icol_off_in_t=icol_off_in_t,
        icol_off_t=icol_off_t,
        total_icols=total_icols,
        total_chunks=total_chunks,
    )

    in_maps = []
    for k in range(N_CORES):
        sel, gid = per_core_sorted[k]
        cnts = counts_all[k].reshape(-1)
        group_start = np.zeros(N_TILES * NB, np.int64)
        group_start[1:] = np.cumsum(cnts)[:-1]
        pos = np.arange(len(sel)) - group_start[gid]  # pos within (t, b)
        t_of = gid // NB
        b_of = gid % NB

        # idx array [16, total_icols] -> replicate to 128 partitions
        idx16 = np.zeros((16, total_icols), np.int16)
        icol = icol_off_t[t_of] + icol_off_in_t[t_of, b_of] + pos // 16
        idx16[pos % 16, icol] = (edge_src[sel] - b_of * B).astype(np.int16)
        idxm = np.ascontiguousarray(np.tile(idx16, (8, 1)))

        # ld array [128, total_chunks] fp16, init -1 (pad -> zero one-hot)
        ldm = np.full((P, total_chunks), -1.0, np.float32)
        ld_sorted = (edge_dst[sel] - k * NPC) % P
        ccol = chunk_off_t[t_of] + slot_off_tb[t_of, b_of] // P + pos // P
        ldm[pos % P, ccol] = ld_sorted.astype(np.float32)

        # per-partition dst norm per tile [128, N_TILES]
        nk = norm[k * NPC : (k + 1) * NPC]
        full = np.zeros(N_TILES * P, np.float32)
        full[: len(nk)] = nk
        nd = full.reshape(N_TILES, P).T.copy()

        in_maps.append(
            {
                "h16": h16,
                "idxm": idxm,
                "ldm": np.ascontiguousarray(ldm),
                "wt": wt16,
                "iotam": iota16,
                "normd": np.ascontiguousarray(nd),
                "resid": np.ascontiguousarray(
                    features[k * NPC : (k + 1) * NPC]
                ),
            }
        )
    return in_maps, layout


def _build_program(layout):
    f32 = mybir.dt.float32
    f16 = mybir.dt.float16
    i16 = mybir.dt.int16
    n_tb = layout["n_tb"]
    c_tb = layout["c_tb"]
    C_t = layout["C_t"]
    Cmax = layout["Cmax"]
    slot_off_tb = layout["slot_off_tb"]
    chunk_off_t = layout["chunk_off_t"]
    icols_tb = layout["icols_tb"]
    icol_off_in_t = layout["icol_off_in_t"]
    icol_off_t = layout["icol_off_t"]

    nc = bacc.Bacc(num_swdge_queues=4)
    h16 = nc.declare_dram_parameter("h16", [N_NODES, P], f16, isOutput=False)
    idxm = nc.declare_dram_parameter(
        "idxm", [P, layout["total_icols"]], i16, isOutput=False
    )
    ldm = nc.declare_dram_parameter(
        "ldm", [P, layout["total_chunks"]], f32, isOutput=False
    )
    wt = nc.declare_dram_parameter("wt", [P, P], f16, isOutput=False)
    iotam = nc.declare_dram_parameter("iotam", [P, P], f16, isOutput=False)
    normd = nc.declare_dram_parameter("normd", [P, N_TILES], f32, isOutput=False)
    resid = nc.declare_dram_parameter("resid", [NPC, P], f32, isOutput=False)
    out = nc.declare_dram_parameter("out", [NPC, P], f32, isOutput=True)

    with TileContext(nc) as tc:
        with (
            tc.tile_pool(name="const", bufs=1) as constp,
            tc.tile_pool(name="meta", bufs=4) as metap,
            tc.tile_pool(name="x", bufs=X_BUFS) as xp,
            tc.tile_pool(name="s", bufs=8) as sp,
            tc.tile_pool(name="zps", bufs=2, space="PSUM") as zpsp,
            tc.tile_pool(name="yps", bufs=2, space="PSUM") as ypsp,
            tc.tile_pool(name="post", bufs=3) as postp,
        ):
            wt_sb = constp.tile([P, P], f16)
            nc.sync.dma_start(out=wt_sb[:], in_=wt[:, :])
            iota_sb = constp.tile([P, P], f16)
            nc.sync.dma_start(out=iota_sb[:], in_=iotam[:, :])
            normd_sb = constp.tile([P, N_TILES], f32)
            nc.sync.dma_start(out=normd_sb[:], in_=normd[:, :])

            for t in range(N_TILES):
                Ct = int(C_t[t])
                icols = int(icols_t_of(layout, t))
                mt_i = metap.tile([P, max(icols, 1)], i16, tag="mi")
                ic0 = int(icol_off_t[t])
                nc.sync.dma_start(
                    out=mt_i[:, :icols], in_=idxm[:, ic0 : ic0 + icols]
                )
                mt_ld = metap.tile([P, Ct], f32, tag="mld")
                cc0 = int(chunk_off_t[t])
                nc.sync.dma_start(out=mt_ld[:], in_=ldm[:, cc0 : cc0 + Ct])

                X_full = xp.tile([P, Cmax * P], f16, tag="X")
                X = X_full[:, : Ct * P]
                if t < X_BUFS:
                    # first rotation of each X buffer: clear so un-gathered
                    # pad slots can't hold NaN bit patterns (0*NaN -> NaN
                    # would poison the one-hot matmul)
                    nc.vector.memzero(X_full[:])
                for b in range(NB):
                    n_idx = int(n_tb[t, b])
                    if n_idx == 0:
                        continue
                    # slot_off_tb is in slots == col offset (128 elems/chunk,
                    # 128 slots/chunk, so chunk_off*P == slot_off)
                    so = int(slot_off_tb[t, b])
                    cb = int(c_tb[t, b])
                    iol = int(icol_off_in_t[t, b])
                    icb = int(icols_tb[t, b])
                    lo = b * B
                    hi = min((b + 1) * B, N_NODES)
                    nc.gpsimd.dma_gather(
                        out_ap=X[:, so : so + cb * P].rearrange(
                            "p (c e) -> p c e", e=P
                        ),
                        in_ap=h16[lo:hi, :],
                        idxs_ap=mt_i[:, iol : iol + icb],
                        num_idxs=n_idx,
                        num_idxs_reg=n_idx,
                        elem_size=P,
                        single_packet=False,
                        queue_num=b,
                    )

                z_ps = zpsp.tile([P, P], f32)
                for c in range(Ct):
                    S = sp.tile([P, P], f16, tag="S")
                    nc.vector.tensor_scalar(
                        out=S[:],
                        in0=iota_sb[:],
                        scalar1=mt_ld[:, c : c + 1],
                        scalar2=None,
                        op0=mybir.AluOpType.is_equal,
                    )
                    nc.tensor.matmul(
                        out=z_ps[:],
                        lhsT=X[:, c * P : (c + 1) * P],
                        rhs=S[:],
                        start=(c == 0),
                        stop=(c == Ct - 1),
                    )

                zT_sb = postp.tile([P, P], f16, tag="zT")
                nc.scalar.copy(out=zT_sb[:], in_=z_ps[:])
                y_ps = ypsp.tile([P, P], f32)
                nc.tensor.matmul(
                    out=y_ps[:], lhsT=zT_sb[:], rhs=wt_sb[:], start=True,
                    stop=True,
                )

                rows = P if t < N_TILES - 1 else ROWS_LAST
                y_sb = postp.tile([P, P], f32, tag="y")
                nc.scalar.activation(
                    out=y_sb[:],
                    in_=y_ps[:],
                    func=mybir.ActivationFunctionType.Relu,
                    scale=normd_sb[:, t : t + 1],
                )
                res_sb = postp.tile([P, P], f32, tag="res")
                nc.sync.dma_start(
                    out=res_sb[:rows], in_=resid[t * P : t * P + rows, :]
                )
                o_sb = postp.tile([P, P], f32, tag="o")
                nc.vector.tensor_add(
                    out=o_sb[:rows], in0=y_sb[:rows], in1=res_sb[:rows]
                )
                nc.sync.dma_start(
                    out=out[t * P : t * P + rows, :], in_=o_sb[:rows]
                )
    nc.finalize()
    return nc


def icols_t_of(layout, t):
    return int(layout["icols_tb"][t].sum())


def _run(features, W, edge_src, edge_dst, trace=False, **spmd_kwargs):
    in_maps, layout = _prepare(features, W, edge_src, edge_dst)
    nc = _build_program(layout)
    br = run_bass_kernel_spmd(
        nc, in_maps, core_ids=list(range(N_CORES)), trace=trace, **spmd_kwargs
    )
    outs = [r["out"] for r in br.results]
    full = np.concatenate(outs, axis=0).astype(np.float32)
    return full, br


def kernel(features, W, edge_src, edge_dst):
    out, _ = _run(features, W, edge_src, edge_dst, trace=False)
    return out
